# revision 11
# baseline (speedup 1.0000x reference)
"""GAT (graph attention) layer on 8 TRN2 NeuronCores — Bass/Tile kernel.

Sharding: destination-node dim i is split across the 8 cores (256 rows
each).  Wh and params are replicated; softmax is over j within a row so
no collective is needed.

Math (per core, rows i in its shard):
  Wh = h @ W                                  [2048, 8, 64]
  e_i[i,h] = <Wh[i,h,:], a_i[h,:]> ;  e_j[j,h] likewise
  P^T[j,(h,i)] = adj[i,j] * exp(leaky_relu(e_i[h,i] + e_j[j,h]))
  out[i,(h,d)] = elu( (P^T.T @ [Wh_h | 1])[:, :64] / denom )

The logits tile y[j,(h,i)] = e_i + e_j is RANK-9: one K=9 matmul per
j-tile materializes all 8 heads at once (rows 0-7 of lhsT are e_j^T,
row 8 is ones; rhs rows 0-7 are head indicators, row 8 is e_i flat).
A ones-column appended to Wh gives the softmax denominator for free.
"""

import dataclasses
import sys

import numpy as np

sys.path.insert(0, "/opt/trn_rl_repo")

N = 2048
F_IN = 768
F_OUT = 64
H = 8
ALPHA = 0.2
NCORES = 8
NL = N // NCORES          # 256 local rows per core
KT = F_IN // 128          # 6 k-tiles
NT = N // 128             # 16 n/j tiles
FH = F_OUT * H            # 512
FW = FH + 2 * H           # 528: [W | wa_j | wa_i] folded rhs

# perf knobs
MM_DT = "float32"        # dtype for big matmuls (float32r = full-rate fp32)
MASK_ZERO_STRIDE = True
DEBUG = False   # one [128,2048] mul vs 8 per-head muls

_CACHE = {}


def _build():
    import concourse.bacc as bacc
    import concourse.mybir as mybir
    from concourse.tile import TileContext

    f32 = mybir.dt.float32
    mmdt = getattr(mybir.dt, MM_DT)
    AF = mybir.ActivationFunctionType
    OP = mybir.AluOpType

    nc = bacc.Bacc("TRN2", target_bir_lowering=False, debug=False,
                   num_devices=NCORES)

    h_d = nc.declare_dram_parameter("h", [N, F_IN], f32, isOutput=False)
    hl_d = nc.declare_dram_parameter("h_local", [NL, F_IN], f32, isOutput=False)
    adjT_d = nc.declare_dram_parameter("adjT", [N, NL], f32, isOutput=False)
    W_d = nc.declare_dram_parameter("W", [F_IN, FH], f32, isOutput=False)
    ai_d = nc.declare_dram_parameter("a_i", [1, FH], f32, isOutput=False)
    aj_d = nc.declare_dram_parameter("a_j", [1, FH], f32, isOutput=False)
    out_d = nc.declare_dram_parameter("out", [NL, FH], f32, isOutput=True)
    if DEBUG:
        dbg_rhs = nc.declare_dram_parameter("dbg_rhs", [H + 1, H * NL], f32, isOutput=True)
        dbg_ejt = nc.declare_dram_parameter("dbg_ejt", [H + 1, N], f32, isOutput=True)
        dbg_L = nc.declare_dram_parameter("dbg_L", [128, H * NL], f32, isOutput=True)
        dbg_E = nc.declare_dram_parameter("dbg_E", [128, H * NL], f32, isOutput=True)
        dbg_dn = nc.declare_dram_parameter("dbg_dn", [128, NT], f32, isOutput=True)
        dbg_wh = nc.declare_dram_parameter("dbg_wh", [128, H * (F_OUT + 1)], f32, isOutput=True)

    def mm(ap):
        return ap.bitcast(mmdt) if mmdt != f32 else ap

    with TileContext(nc) as tc:
        with tc.tile_pool(name="persist", bufs=1) as pp:
            ident = pp.tile([128, 128], f32)
            W_sb = pp.tile([128, KT, FW], f32)
            hT = pp.tile([128, KT * N], f32)
            Wh_aug = pp.tile([128, NT, H, F_OUT + 1], f32)
            adjT_sb = pp.tile([128, NT, NL], f32)
            ejT_aug = pp.tile([H + 1, N], f32)
            rhs_sb = pp.tile([H + 1, H * NL], f32)
            eiT_sb = pp.tile([H, NL], f32)
            hlT = pp.tile([128, KT * NL], f32)
            ai_bc = pp.tile([128, FH], f32)
            aj_bc = pp.tile([128, FH], f32)
            ones_row = pp.tile([1, 128], f32)
            hp_sb = pp.tile([128, 2, FH], f32)
            mn_sb = pp.tile([128, 2, FH], f32)
            em_sb = pp.tile([128, 2, FH], f32)
            out_sb = pp.tile([128, 2, FH], f32)
            r_sb = pp.tile([128, NT], f32)
            dn_sb = pp.tile([128, NT], f32)
            zs_row = pp.tile([1, 512], f32)

            # ---------- phase 1: params, hT, Wh, e_i/e_j ----------
            with tc.tile_pool(name="ph1", bufs=3) as sp, \
                 tc.tile_pool(name="ph1ps", bufs=2, space="PSUM") as ps, \
                 tc.tile_pool(name="ph1ps1", bufs=1, space="PSUM") as ps1:

                # identity for PE transposes
                io_t = sp.tile([128, 128], mybir.dt.int32, tag="iota")
                nc.gpsimd.iota(io_t[:], pattern=[[-1, 128]], base=0,
                               channel_multiplier=1)
                nc.vector.tensor_scalar(ident[:], io_t[:], 0, None,
                                        OP.is_equal)
                nc.gpsimd.memset(ones_row[:], 1.0)
                nc.vector.memset(zs_row[:], 0.0)

                # params in
                for k in range(KT):
                    nc.sync.dma_start(out=W_sb[:, k, 0:FH],
                                      in_=W_d[k * 128:(k + 1) * 128, :])
                a_t = sp.tile([1, FH], f32, tag="a")
                nc.sync.dma_start(out=a_t[:], in_=ai_d[:])
                a2_t = sp.tile([1, FH], f32, tag="a")
                nc.sync.dma_start(out=a2_t[:], in_=aj_d[:])
                for jt in range(NT):
                    nc.sync.dma_start(
                        out=adjT_sb[:, jt, :],
                        in_=adjT_d[jt * 128:(jt + 1) * 128, :])

                # broadcast a_i/a_j to 128 partitions (K=1 matmul)
                for src, dst in ((a_t, ai_bc), (a2_t, aj_bc)):
                    ps_b = ps1.tile([128, FH], f32, tag="abc")
                    nc.tensor.matmul(ps_b[:], ones_row[:], src[:],
                                     start=True, stop=True)
                    nc.scalar.copy(out=dst[:], in_=ps_b[:])

                # fold wa_j / wa_i into W_sb cols [FH:FH+8], [FH+8:FH+16]
                for k in range(KT):
                    for (bc, off) in ((aj_bc, FH), (ai_bc, FH + H)):
                        t_t = sp.tile([128, FH], f32, tag="wtmp")
                        nc.vector.tensor_tensor(t_t[:], W_sb[:, k, 0:FH],
                                                bc[:], OP.mult)
                        nc.vector.tensor_reduce(
                            W_sb[:, k, off:off + H],
                            t_t[:].rearrange("p (h d) -> p h d", h=H),
                            mybir.AxisListType.X, OP.add)

                # transpose h -> hT   (96 PE transposes)
                for nt in range(NT):
                    h_t = sp.tile([128, F_IN], f32, tag="h")
                    nc.sync.dma_start(out=h_t[:],
                                      in_=h_d[nt * 128:(nt + 1) * 128, :])
                    for k in range(KT):
                        ps_t = ps.tile([128, 128], f32, tag="tp")
                        nc.tensor.transpose(ps_t[:],
                                            h_t[:, k * 128:(k + 1) * 128],
                                            ident[:])
                        dst = hT[:, k * N + nt * 128: k * N + nt * 128 + 128]
                        if k % 2 == 0:
                            nc.vector.tensor_copy(dst, ps_t[:])
                        else:
                            nc.scalar.copy(out=dst, in_=ps_t[:])

                # transpose h_local -> hlT
                for lt in range(NL // 128):
                    h_t = sp.tile([128, F_IN], f32, tag="h")
                    nc.sync.dma_start(out=h_t[:],
                                      in_=hl_d[lt * 128:(lt + 1) * 128, :])
                    for k in range(KT):
                        ps_t = ps.tile([128, 128], f32, tag="tp")
                        nc.tensor.transpose(ps_t[:],
                                            h_t[:, k * 128:(k + 1) * 128],
                                            ident[:])
                        nc.vector.tensor_copy(
                            hlT[:, k * NL + lt * 128: k * NL + lt * 128 + 128],
                            ps_t[:])

                # ones plane of Wh_aug; ejT row 8 = ones (rows 0-7
                # overwritten later); rhs rows 0-7 = head indicators via
                # iota (p == block) — compute APs must start at partition 0
                nc.gpsimd.memset(Wh_aug[:, :, :, F_OUT:F_OUT + 1], 1.0)
                nc.vector.memset(ejT_aug[:], 1.0)
                io_r = sp.tile([H + 1, H, NL], mybir.dt.int32, tag="iotar")
                nc.gpsimd.iota(io_r[:], pattern=[[-1, H], [0, NL]], base=0,
                               channel_multiplier=1)
                nc.vector.tensor_scalar(
                    rhs_sb[:].rearrange("p (h i) -> p h i", h=H),
                    io_r[:], 0, None, OP.is_equal)

                # Wh (+ folded e_j, e_i) = hT.T @ [W | wa_j | wa_i]
                for nt in range(NT):
                    ps_w = ps.tile([128, FW], f32, tag="wh")
                    for k in range(KT):
                        lhs = hT[:, k * N + nt * 128: k * N + nt * 128 + 128]
                        nc.tensor.matmul(ps_w[:, 0:512], mm(lhs),
                                         mm(W_sb[:, k, 0:512]),
                                         start=(k == 0), stop=(k == KT - 1))
                        nc.tensor.matmul(ps_w[:, 512:FW], mm(lhs),
                                         mm(W_sb[:, k, 512:FW]),
                                         start=(k == 0), stop=(k == KT - 1))
                    # evac Wh rows into per-head 65-stride layout
                    nc.scalar.copy(
                        out=Wh_aug[:, nt, :, 0:F_OUT],
                        in_=ps_w[:, 0:FH].rearrange("p (h d) -> p h d", h=H))
                    # e_j tile -> SBUF -> transpose -> ejT rows 0-7
                    ej_t = sp.tile([128, H], f32, tag="ej")
                    nc.vector.tensor_copy(ej_t[:], ps_w[:, FH:FH + H])
                    ps_e = ps1.tile([H, 128], f32, tag="ejt")
                    nc.tensor.transpose(ps_e[:], ej_t[:], ident[:])
                    nc.vector.tensor_copy(
                        ejT_aug[0:H, nt * 128:(nt + 1) * 128], ps_e[:])

                # e_i from h_local
                for lt in range(NL // 128):
                    ps_w = ps.tile([128, FW], f32, tag="wh")
                    for k in range(KT):
                        lhs = hlT[:, k * NL + lt * 128: k * NL + lt * 128 + 128]
                        nc.tensor.matmul(ps_w[:, 512:FW], mm(lhs),
                                         mm(W_sb[:, k, 512:FW]),
                                         start=(k == 0), stop=(k == KT - 1))
                    ei_t = sp.tile([128, H], f32, tag="ej")
                    nc.vector.tensor_copy(ei_t[:], ps_w[:, FH + H:FW])
                    ps_e = ps1.tile([H, 128], f32, tag="ejt")
                    nc.tensor.transpose(ps_e[:], ei_t[:], ident[:])
                    nc.vector.tensor_copy(
                        eiT_sb[:, lt * 128:(lt + 1) * 128], ps_e[:])
                # flatten eiT rows into rhs row 8 (f = h*NL + i)
                for hh in range(H):
                    nc.sync.dma_start(
                        out=rhs_sb[H:H + 1, hh * NL:(hh + 1) * NL],
                        in_=eiT_sb[hh:hh + 1, :])

            # ---------- phase 2: main loop over j-tiles ----------
            with tc.tile_pool(name="ebuf", bufs=2) as eb, \
                 tc.tile_pool(name="psy", bufs=1, space="PSUM") as pyp, \
                 tc.tile_pool(name="psagg", bufs=1, space="PSUM") as pap:

                ps_agg = pap.tile([128, NT, F_OUT + 1], f32)

                # start=True clears the WHOLE psum bank, so 16 interleaved
                # accumulation groups sharing banks would wipe each other.
                # Zero each bank once via dummy matmuls; groups accumulate
                # with start=False on top.
                agg_flat = ps_agg[:].rearrange("p g d -> p (g d)")
                tot = NT * (F_OUT + 1)
                off = 0
                while off < tot:
                    w = min(512, tot - off)
                    nc.tensor.matmul(agg_flat[:, off:off + w],
                                     zs_row[0:1, 0:128],
                                     zs_row[0:1, 0:w],
                                     start=True, stop=False,
                                     skip_group_check=True)
                    off += w

                for jt in range(NT):
                    ps_y = pyp.tile([128, H * NL], f32, tag="y")
                    lhs = ejT_aug[:, jt * 128:(jt + 1) * 128]
                    for q in range(4):
                        nc.tensor.matmul(ps_y[:, q * 512:(q + 1) * 512],
                                         mm(lhs),
                                         mm(rhs_sb[:, q * 512:(q + 1) * 512]),
                                         start=True, stop=True)
                    # exact leaky_relu: max(alpha*y, y).  ACT Lrelu has a
                    # fixed 0.01 slope table (alpha arg ignored), and DVE ops
                    # may read only one PSUM input — so ACT makes the scaled
                    # copy and DVE maxes it against PSUM.
                    t_t = eb.tile([128, H * NL], f32, tag="Ls")
                    nc.scalar.mul(out=t_t[:], in_=ps_y[:], mul=ALPHA)
                    L_t = eb.tile([128, H * NL], f32, tag="L")
                    nc.vector.tensor_tensor(L_t[:], t_t[:], ps_y[:], OP.max)
                    E_t = eb.tile([128, H * NL], f32, tag="E")
                    nc.scalar.activation(E_t[:], L_t[:], AF.Exp)
                    # mask by adjacency (same mask for all 8 heads)
                    if MASK_ZERO_STRIDE:
                        base = adjT_sb[:, jt, :]
                        rep = dataclasses.replace(
                            base, ap=[list(base.ap[0]), [0, H],
                                      list(base.ap[1])])
                        nc.vector.tensor_tensor(
                            E_t[:].rearrange("p (h i) -> p h i", h=H),
                            E_t[:].rearrange("p (h i) -> p h i", h=H),
                            rep, OP.mult)
                    else:
                        for hh in range(H):
                            nc.vector.tensor_tensor(
                                E_t[:, hh * NL:(hh + 1) * NL],
                                E_t[:, hh * NL:(hh + 1) * NL],
                                adjT_sb[:, jt, :], OP.mult)
                    if DEBUG and jt == 0:
                        nc.sync.dma_start(out=dbg_L[:], in_=L_t[:])
                        nc.sync.dma_start(out=dbg_E[:], in_=E_t[:])
                    # aggregation matmuls
                    for hh in range(H):
                        for ih in range(2):
                            g = hh * 2 + ih
                            nc.tensor.matmul(
                                ps_agg[:, g, :],
                                mm(E_t[:, hh * NL + ih * 128:
                                       hh * NL + ih * 128 + 128]),
                                mm(Wh_aug[:, jt, hh, :]),
                                start=False, stop=(jt == NT - 1),
                                skip_group_check=True)

                # ---------- finalize: normalize + ELU + store ----------
                if DEBUG:
                    nc.sync.dma_start(out=dbg_rhs[:], in_=rhs_sb[:])
                    nc.sync.dma_start(out=dbg_ejt[:], in_=ejT_aug[:])
                    nc.sync.dma_start(
                        out=dbg_wh[:],
                        in_=Wh_aug[:, 0, :, :].rearrange("p h d -> p (h d)"))
                nc.vector.tensor_copy(dn_sb[:], ps_agg[:, :, F_OUT])
                if DEBUG:
                    nc.sync.dma_start(out=dbg_dn[:], in_=dn_sb[:])
                nc.vector.reciprocal(r_sb[:], dn_sb[:])
                for hh in range(H):
                    for ih in range(2):
                        g = hh * 2 + ih
                        nc.vector.tensor_scalar(
                            hp_sb[:, ih, hh * F_OUT:(hh + 1) * F_OUT],
                            ps_agg[:, g, 0:F_OUT],
                            r_sb[:, g:g + 1], None, OP.mult)
                nc.vector.tensor_scalar(mn_sb[:], hp_sb[:], 0.0, None, OP.min)
                nc.scalar.activation(em_sb[:], mn_sb[:], AF.Exp)
                nc.vector.scalar_tensor_tensor(out_sb[:], em_sb[:], -1.0,
                                               hp_sb[:], OP.add, OP.max)
                for ih in range(2):
                    nc.sync.dma_start(out=out_d[ih * 128:(ih + 1) * 128, :],
                                      in_=out_sb[:, ih, :])

    nc.compile()
    return nc


def kernel(h, adj, W, a):
    from concourse.bass_utils import run_bass_kernel_spmd

    if "nc" not in _CACHE:
        _CACHE["nc"] = _build()
    nc = _CACHE["nc"]

    h = np.ascontiguousarray(h, dtype=np.float32)
    adj = np.ascontiguousarray(adj, dtype=np.float32)
    W = np.ascontiguousarray(W, dtype=np.float32)
    a = np.ascontiguousarray(a, dtype=np.float32)
    a_i = np.ascontiguousarray(a[0, :, :F_OUT].reshape(1, FH))
    a_j = np.ascontiguousarray(a[0, :, F_OUT:].reshape(1, FH))

    in_maps = []
    for c in range(NCORES):
        sl = slice(c * NL, (c + 1) * NL)
        in_maps.append({
            "h": h,
            "h_local": np.ascontiguousarray(h[sl]),
            "adjT": np.ascontiguousarray(adj[sl].T),
            "W": W,
            "a_i": a_i,
            "a_j": a_j,
        })
    res = run_bass_kernel_spmd(nc, in_maps, list(range(NCORES)),
                               trace=bool(_CACHE.get("trace")))
    _CACHE["last"] = res
    return np.concatenate([res.results[c]["out"] for c in range(NCORES)],
                          axis=0)
